# revision 14
# baseline (speedup 1.0000x reference)
"""Bezier surface fitter as a sharded matmul on 8 TRN2 NeuronCores.

out[b,c,h,w] = sum_{p,q} basis[h*w, p, q] * K[b, c, p, q]

Strategy (per sharding hint): shard h*w across the 8 cores, replicate K.

Fast path (always taken for a genuine Bezier basis): the basis is separable,
basis[(i,j),p,q] = F[i,p]*G[j,q], verified elementwise on the host.  The
host folds F into K (A[bc,i,q] = sum_p F[i,p]K[bc,p,q], a 128x16x16-sized
job) and each core expands its 64 i-rows: out[bc, i*W+j] = sum_q A[bc,i,q]
G[j,q] — one K=32 f16 matmul per i-row with lhsT=[Ah;Al], rhs=[Gh;Gh]
(exact in A; only G carries f16 rounding).  Results are written to DRAM as
f16 (harness tolerance 2e-2, f16 adds ~3e-4) halving the output traffic,
and upcast to f32 on the host.  Steady state is a balanced ridge: PE
~64x512 f16 columns, DVE+ACT PSUM->SBUF cast copies, and ~8.4 MB of
output DMA all land within ~15% of each other.

General path (fallback if the basis is not separable): plain tiled matmul
OUT[bc, n] = KF[bc, 256] @ BF[n, 256]^T with n sharded across cores.
"""

import os

import numpy as np

import concourse.bass as bass
import concourse.mybir as mybir
from concourse import bacc
from concourse.bass_utils import run_bass_kernel_spmd
from concourse.tile import TileContext

N_CORES = 8
B, C, H, W, M1, N1 = 8, 16, 512, 512, 16, 16
BC = B * C            # 128
KDIM = M1 * N1        # 256
HW = H * W            # 262144
SHARD = HW // N_CORES  # 32768

NT = 2048             # output columns per outer tile (psum tile = 4 banks)
MM_N = 512            # moving free dim per matmul (one psum bank of f32)
KCHUNKS = KDIM // 128  # 2

MM_DTYPE = mybir.dt.float32  # switchable: float32 | float32r | bfloat16

LAST_RESULT = None  # BassKernelResults of the most recent run (for test harness)


def _build_nc(mm_dtype=None, nt=None, b_bufs=4, o_bufs=4, p_bufs=2, repeats=1, _alt=False):
    mm_dtype = MM_DTYPE if mm_dtype is None else mm_dtype
    io_dtype = mm_dtype if mm_dtype == mybir.dt.float32r else mybir.dt.float32
    global NT
    NT_save = NT
    if nt is not None:
        NT = nt
    nc = bacc.Bacc()
    kt = nc.declare_dram_parameter("kt", [KDIM, BC], io_dtype, isOutput=False)
    if _alt:  # test-harness only: alternate weights across repeats
        kt2 = nc.declare_dram_parameter("kt2", [KDIM, BC], io_dtype, isOutput=False)
    bt = nc.declare_dram_parameter("bt", [KDIM, SHARD], io_dtype, isOutput=False)
    out = nc.declare_dram_parameter("out", [BC, SHARD], mybir.dt.float32, isOutput=True)

    n_tiles = SHARD // NT
    with TileContext(nc) as tc:
        with (
            tc.tile_pool(name="kpool", bufs=1) as kpool,
            tc.tile_pool(name="bpool", bufs=b_bufs) as bpool,
            tc.tile_pool(name="opool", bufs=o_bufs) as opool,
            tc.tile_pool(name="ppool", bufs=p_bufs, space="PSUM") as ppool,
        ):
            ktile = kpool.tile([128, KCHUNKS * BC], io_dtype)
            nc.sync.dma_start(
                out=ktile[:, :].rearrange("p (c m) -> p c m", c=KCHUNKS),
                in_=kt[:, :].rearrange("(c p) m -> p c m", p=128),
            )
            if _alt:
                ktile2 = kpool.tile([128, KCHUNKS * BC], io_dtype)
                nc.sync.dma_start(
                    out=ktile2[:, :].rearrange("p (c m) -> p c m", c=KCHUNKS),
                    in_=kt2[:, :].rearrange("(c p) m -> p c m", p=128),
                )
            for _rep in range(repeats):
                kt_use = ktile2 if (_alt and _rep % 2) else ktile
                for t in range(n_tiles):
                    btile = bpool.tile([128, KCHUNKS * NT], io_dtype)
                    nc.sync.dma_start(
                        out=btile[:, :].rearrange("p (c n) -> p c n", c=KCHUNKS),
                        in_=bt[:, :].rearrange("(c p) n -> p c n", p=128)[
                            :, :, t * NT : (t + 1) * NT
                        ],
                    )
                    ptile = ppool.tile([128, NT], mybir.dt.float32)
                    for j in range(NT // MM_N):
                        for c in range(KCHUNKS):
                            nc.tensor.matmul(
                                ptile[:, j * MM_N : (j + 1) * MM_N],
                                lhsT=kt_use[:, c * BC : (c + 1) * BC].bitcast(mm_dtype),
                                rhs=btile[
                                    :, c * NT + j * MM_N : c * NT + (j + 1) * MM_N
                                ].bitcast(mm_dtype),
                                start=(c == 0),
                                stop=(c == KCHUNKS - 1),
                            )
                    otile = opool.tile([128, NT], mybir.dt.float32)
                    nc.vector.tensor_copy(otile[:, :], ptile[:, :])
                    nc.sync.dma_start(
                        out=out[:, t * NT : (t + 1) * NT], in_=otile[:, :]
                    )
    NT = NT_save
    nc.finalize()
    return nc


ILOC = H // N_CORES  # 64 rows of the h-grid per core on the fast path
KST = 32             # stacked contraction depth: [Ah; Al] x [Gh; Gh]


def _build_nc_fast(o_bufs=6, p_bufs=8, repeats=1, OG=8, PG=2, _alt=False):
    """Fast path: basis is separable (basis[(i,j),p,q] = F[i,p] * G[j,q]).

    Host precomputes A[bc,i,q] = sum_p F[i,p] * K[bc,p,q]; the device only
    expands out[bc, i*W+j] = sum_q A[bc,i,q] * G[j,q] — then streams results
    out.  A is shipped as a float16 hi/lo split pair STACKED along the
    contraction dim: lhsT = [Ah; Al] (K=32), rhs = [Gh; Gh], so one f16
    matmul per output block computes (Ah+Al)*Gh = A*Gh exactly in A; the
    only input rounding is G's f16 quantization (~2^-11 relative).
    The output is written to DRAM as float16 (the harness tolerance is
    2e-2; f16 output rounding adds ~3e-4) — this HALVES the output HBM
    traffic, which is the binding roofline of the whole kernel.
    Per-core inputs: ast = stacked A^T slice [32, ILOC*128] f16,
                     bst = stacked G^T [32, W] f16.
    """
    f16 = mybir.dt.float16
    nc = bacc.Bacc()
    ast = nc.declare_dram_parameter("ast", [KST, ILOC * BC], f16, isOutput=False)
    bst = nc.declare_dram_parameter("bst", [KST, W], f16, isOutput=False)
    if _alt:  # test-harness only: alternate rhs across repeats so no repeat
        bst2 = nc.declare_dram_parameter("bst2", [KST, W], f16, isOutput=False)
    out = nc.declare_dram_parameter("out", [BC, SHARD], f16, isOutput=True)

    with TileContext(nc) as tc:
        with (
            tc.tile_pool(name="cpool", bufs=1) as cpool,
            tc.tile_pool(name="opool", bufs=o_bufs) as opool,
            tc.tile_pool(name="ppool", bufs=p_bufs // PG, space="PSUM") as ppool,
        ):
            bsttile = cpool.tile([KST, W], f16)
            nc.sync.dma_start(out=bsttile[:, :], in_=bst[:, :])
            if _alt:
                bsttile2 = cpool.tile([KST, W], f16)
                nc.sync.dma_start(out=bsttile2[:, :], in_=bst2[:, :])
            asttile = cpool.tile([KST, ILOC * BC], f16)
            # progressive chunks: a tiny first chunk ungates the first
            # matmuls almost immediately; a few big tails amortize the
            # ~0.6 us per-DMA-instruction overhead that otherwise
            # dominates the kernel head (16 uniform chunks cost ~11 us).
            # Chunks go out on different engines' DGE queues so their
            # descriptor-gen + transfers overlap instead of serializing
            # on the SP queue.
            bounds = [0, 4, 16, 40, 64]
            engines = [nc.sync, nc.scalar, nc.sync, nc.scalar]
            for (c0, c1), eng in zip(zip(bounds[:-1], bounds[1:]), engines):
                eng.dma_start(
                    out=asttile[:, c0 * BC : c1 * BC],
                    in_=ast[:, c0 * BC : c1 * BC],
                )
            for _rep in range(repeats):
                bt_use = bsttile2 if (_alt and _rep % 2) else bsttile
                for g in range(ILOC // OG):
                    otile = opool.tile([128, OG * W], f16)
                    for s2 in range(OG // PG):
                        ptile = ppool.tile([128, PG * W], mybir.dt.float32)
                        for u in range(PG):
                            il = g * OG + s2 * PG + u
                            nc.tensor.matmul(
                                ptile[:, u * W : (u + 1) * W],
                                lhsT=asttile[:, il * BC : (il + 1) * BC],
                                rhs=bt_use[:, :],
                                start=True,
                                stop=True,
                            )
                        # split the PSUM->SBUF copy (with f32->f16 cast)
                        # across VectorE and ScalarE in proportion to their
                        # fp32-input rates (123 vs 154 G elem/s — DVE's 2x
                        # 16-bit mode does not apply to PSUM/f32 reads)
                        vshare = (PG * W * 9) // 20
                        nc.vector.tensor_copy(
                            otile[:, s2 * PG * W : s2 * PG * W + vshare],
                            ptile[:, :vshare],
                        )
                        nc.scalar.copy(
                            otile[:, s2 * PG * W + vshare : (s2 + 1) * PG * W],
                            ptile[:, vshare:],
                        )
                    # two half-tile stores: spreads work over more DMA
                    # engine slots and lets the first half fly while the
                    # second half's copies finish
                    half = OG * W // 2
                    nc.sync.dma_start(
                        out=out[:, g * OG * W : g * OG * W + half],
                        in_=otile[:, :half],
                    )
                    nc.sync.dma_start(
                        out=out[:, g * OG * W + half : (g + 1) * OG * W],
                        in_=otile[:, half:],
                    )
    nc.finalize()
    return nc


def _try_separate(basis4):
    """If basis[(i,j),p,q] == F[i,p] * G[j,q] (to fp32 accuracy), return
    (F, G) as float64 arrays; else None.  Exact-by-construction check: the
    factorization is verified elementwise against the provided data."""
    S = basis4.sum(axis=(1, 3), dtype=np.float64)  # [H, M1] = F * sum(G)
    T = basis4.sum(axis=(0, 2), dtype=np.float64)  # [W, N1] = G * sum(F)
    tot = float(S.sum())
    if not np.isfinite(tot) or abs(tot) < 1e-30:
        return None
    F = S
    G = T / tot
    scale = float(np.max(np.abs(basis4)))
    if scale == 0.0 or not np.isfinite(scale):
        return None
    # chunked elementwise verification of the reconstruction.  A truly
    # separable f32 tensor reconstructs to ~3e-8 * scale (f32 rounding);
    # 1e-6 leaves margin while rejecting anything meaningfully non-rank-1.
    for i0 in range(0, H, 64):
        rec = np.einsum(
            "ip,jq->ijpq", F[i0 : i0 + 64], G, optimize=True
        ).astype(np.float32)
        err = np.max(np.abs(rec - basis4[i0 : i0 + 64]))
        if not (err <= 1e-6 * scale):
            return None
    return F, G


def kernel(K: np.ndarray, basis: np.ndarray) -> np.ndarray:
    global LAST_RESULT
    K = np.ascontiguousarray(np.asarray(K, dtype=np.float32))
    basis = np.asarray(basis, dtype=np.float32)

    force = os.environ.get("BASS_KERNEL_FORCE", "")  # "", "fast", "general"
    fact = None
    if force != "general":
        fact = _try_separate(basis.reshape(H, W, M1, N1))

    trace = os.environ.get("BASS_KERNEL_TRACE", "0") == "1"
    core_ids = list(range(N_CORES))

    if fact is not None:
        try:
            return _run_fast(K, fact, core_ids, trace)
        except Exception:
            pass  # graceful degradation: fall through to the general path
    return _run_general(K, basis, core_ids, trace)


def _build_nc_noop():
    """Timing-harness helper: a NEFF with the fast path's exact I/O
    signature but ~zero device work (one tiny load + one tiny store).
    Launching it interleaved with the real reps=1 kernel and taking the
    paired wall-clock difference measures the real kernel's full
    single-exec device time directly on hardware — head, steady state,
    and drain — with the axon dispatch floor cancelled."""
    f16 = mybir.dt.float16
    nc = bacc.Bacc()
    ast = nc.declare_dram_parameter("ast", [KST, ILOC * BC], f16, isOutput=False)
    bst = nc.declare_dram_parameter("bst", [KST, W], f16, isOutput=False)
    out = nc.declare_dram_parameter("out", [BC, SHARD], f16, isOutput=True)
    with TileContext(nc) as tc:
        with tc.tile_pool(name="cpool", bufs=1) as cpool:
            t = cpool.tile([KST, W], f16)
            nc.sync.dma_start(out=t[:, :], in_=bst[:, :])
            t2 = cpool.tile([KST, W], f16)
            nc.sync.dma_start(out=t2[:, :], in_=ast[:, :W])
            nc.sync.dma_start(out=out[:KST, :W], in_=t[:, :])
            nc.sync.dma_start(out=out[KST : 2 * KST, :W], in_=t2[:, :])
    nc.finalize()
    return nc


def _pack_fast_inputs(K, fact):
    """Host-side packing for the fast path: per-core input maps."""
    F, G = fact
    # rebalance so both factors are O(1): the f16 hi/lo split loses
    # precision badly when one factor carries a ~512x scale
    c = float(np.max(np.abs(F)))
    F = F / c
    G = G * c
    # A[bc, i, q] = sum_p F[i,p] * K[bc,p,q]
    A = np.einsum(
        "ip,bpq->biq", F, K.reshape(BC, M1, N1).astype(np.float64), optimize=True
    ).astype(np.float32)
    G32 = G.astype(np.float32)
    bh = G32.astype(np.float16)
    bst = np.concatenate([bh.T, bh.T], axis=0)  # [32, W]
    bst = np.ascontiguousarray(bst)
    A_hi = A.astype(np.float16)
    A_lo = (A - A_hi.astype(np.float32)).astype(np.float16)
    in_maps = []
    for i in range(N_CORES):
        sl = slice(i * ILOC, (i + 1) * ILOC)
        aht = A_hi[:, sl, :].transpose(2, 1, 0).reshape(M1, ILOC * BC)
        alt = A_lo[:, sl, :].transpose(2, 1, 0).reshape(M1, ILOC * BC)
        ast = np.ascontiguousarray(
            np.concatenate([aht, alt], axis=0)
        )  # [32, ILOC*BC]
        in_maps.append({"ast": ast, "bst": bst})
    return in_maps


def _run_fast(K, fact, core_ids, trace):
    global LAST_RESULT
    in_maps = _pack_fast_inputs(K, fact)
    nc = _build_nc_fast()
    LAST_RESULT = run_bass_kernel_spmd(nc, in_maps, core_ids=core_ids, trace=trace)
    res = LAST_RESULT.results
    out = np.concatenate(
        [res[i]["out"] for i in range(N_CORES)], axis=1
    )  # [128, HW] f16
    return out.astype(np.float32).reshape(1, B, C, H, W)


def _run_general(K, basis, core_ids, trace):
    global LAST_RESULT
    kt_full = np.ascontiguousarray(K.reshape(BC, KDIM).T)  # [256, 128]
    bflat = basis.reshape(HW, KDIM)
    in_maps = []
    for i in range(N_CORES):
        bt_i = np.ascontiguousarray(
            bflat[i * SHARD : (i + 1) * SHARD].T
        )  # [256, SHARD]
        in_maps.append({"kt": kt_full, "bt": bt_i})
    nc = _build_nc(nt=1024, b_bufs=4, o_bufs=4, p_bufs=2)
    LAST_RESULT = run_bass_kernel_spmd(nc, in_maps, core_ids=core_ids, trace=trace)
    res = LAST_RESULT.results
    out = np.concatenate([res[i]["out"] for i in range(N_CORES)], axis=1)  # [128, HW]
    return out.reshape(1, B, C, H, W)



# revision 16
# speedup vs baseline: 1.2718x; 1.2718x over previous
"""Bezier surface fitter as a sharded matmul on 8 TRN2 NeuronCores.

out[b,c,h,w] = sum_{p,q} basis[h*w, p, q] * K[b, c, p, q]

Strategy (per sharding hint): shard h*w across the 8 cores, replicate K.

Fast path (always taken for a genuine Bezier basis): the basis is separable,
basis[(i,j),p,q] = F[i,p]*G[j,q], verified elementwise on the host.  The
host folds F into K (A[bc,i,q] = sum_p F[i,p]K[bc,p,q], a 128x16x16-sized
job) and each core expands its 64 i-rows: out[bc, i*W+j] = sum_q A[bc,i,q]
G[j,q] — one K=32 f16 matmul per i-row with lhsT=[Ah;Al], rhs=[Gh;Gh]
(exact in A; only G carries f16 rounding).  Results are written to DRAM as
f16 (harness tolerance 2e-2, f16 adds ~3e-4) halving the output traffic,
and upcast to f32 on the host.  Steady state is a balanced ridge: PE
~64x512 f16 columns, DVE+ACT PSUM->SBUF cast copies, and ~8.4 MB of
output DMA all land within ~15% of each other.

General path (fallback if the basis is not separable): plain tiled matmul
OUT[bc, n] = KF[bc, 256] @ BF[n, 256]^T with n sharded across cores.
"""

import os

import numpy as np

import concourse.bass as bass
import concourse.mybir as mybir
from concourse import bacc
from concourse.bass_utils import run_bass_kernel_spmd
from concourse.tile import TileContext

N_CORES = 8
B, C, H, W, M1, N1 = 8, 16, 512, 512, 16, 16
BC = B * C            # 128
KDIM = M1 * N1        # 256
HW = H * W            # 262144
SHARD = HW // N_CORES  # 32768

NT = 2048             # output columns per outer tile (psum tile = 4 banks)
MM_N = 512            # moving free dim per matmul (one psum bank of f32)
KCHUNKS = KDIM // 128  # 2

MM_DTYPE = mybir.dt.float32  # switchable: float32 | float32r | bfloat16

LAST_RESULT = None  # BassKernelResults of the most recent run (for test harness)


def _build_nc(mm_dtype=None, nt=None, b_bufs=4, o_bufs=4, p_bufs=2, repeats=1, _alt=False):
    mm_dtype = MM_DTYPE if mm_dtype is None else mm_dtype
    io_dtype = mm_dtype if mm_dtype == mybir.dt.float32r else mybir.dt.float32
    global NT
    NT_save = NT
    if nt is not None:
        NT = nt
    nc = bacc.Bacc()
    kt = nc.declare_dram_parameter("kt", [KDIM, BC], io_dtype, isOutput=False)
    if _alt:  # test-harness only: alternate weights across repeats
        kt2 = nc.declare_dram_parameter("kt2", [KDIM, BC], io_dtype, isOutput=False)
    bt = nc.declare_dram_parameter("bt", [KDIM, SHARD], io_dtype, isOutput=False)
    out = nc.declare_dram_parameter("out", [BC, SHARD], mybir.dt.float32, isOutput=True)

    n_tiles = SHARD // NT
    with TileContext(nc) as tc:
        with (
            tc.tile_pool(name="kpool", bufs=1) as kpool,
            tc.tile_pool(name="bpool", bufs=b_bufs) as bpool,
            tc.tile_pool(name="opool", bufs=o_bufs) as opool,
            tc.tile_pool(name="ppool", bufs=p_bufs, space="PSUM") as ppool,
        ):
            ktile = kpool.tile([128, KCHUNKS * BC], io_dtype)
            nc.sync.dma_start(
                out=ktile[:, :].rearrange("p (c m) -> p c m", c=KCHUNKS),
                in_=kt[:, :].rearrange("(c p) m -> p c m", p=128),
            )
            if _alt:
                ktile2 = kpool.tile([128, KCHUNKS * BC], io_dtype)
                nc.sync.dma_start(
                    out=ktile2[:, :].rearrange("p (c m) -> p c m", c=KCHUNKS),
                    in_=kt2[:, :].rearrange("(c p) m -> p c m", p=128),
                )
            for _rep in range(repeats):
                kt_use = ktile2 if (_alt and _rep % 2) else ktile
                for t in range(n_tiles):
                    btile = bpool.tile([128, KCHUNKS * NT], io_dtype)
                    nc.sync.dma_start(
                        out=btile[:, :].rearrange("p (c n) -> p c n", c=KCHUNKS),
                        in_=bt[:, :].rearrange("(c p) n -> p c n", p=128)[
                            :, :, t * NT : (t + 1) * NT
                        ],
                    )
                    ptile = ppool.tile([128, NT], mybir.dt.float32)
                    for j in range(NT // MM_N):
                        for c in range(KCHUNKS):
                            nc.tensor.matmul(
                                ptile[:, j * MM_N : (j + 1) * MM_N],
                                lhsT=kt_use[:, c * BC : (c + 1) * BC].bitcast(mm_dtype),
                                rhs=btile[
                                    :, c * NT + j * MM_N : c * NT + (j + 1) * MM_N
                                ].bitcast(mm_dtype),
                                start=(c == 0),
                                stop=(c == KCHUNKS - 1),
                            )
                    otile = opool.tile([128, NT], mybir.dt.float32)
                    nc.vector.tensor_copy(otile[:, :], ptile[:, :])
                    nc.sync.dma_start(
                        out=out[:, t * NT : (t + 1) * NT], in_=otile[:, :]
                    )
    NT = NT_save
    nc.finalize()
    return nc


ILOC = H // N_CORES  # 64 rows of the h-grid per core on the fast path
KST = 32             # stacked contraction depth: [Ah; Al] x [Gh; Gh]


def _build_nc_fast(o_bufs=6, p_bufs=8, repeats=1, OG=8, PG=2, _alt=False):
    """Fast path: basis is separable (basis[(i,j),p,q] = F[i,p] * G[j,q]).

    Host precomputes A[bc,i,q] = sum_p F[i,p] * K[bc,p,q]; the device only
    expands out[bc, i*W+j] = sum_q A[bc,i,q] * G[j,q] — then streams results
    out.  A is shipped as a float16 hi/lo split pair STACKED along the
    contraction dim: lhsT = [Ah; Al] (K=32), rhs = [Gh; Gh], so one f16
    matmul per output block computes (Ah+Al)*Gh = A*Gh exactly in A; the
    only input rounding is G's f16 quantization (~2^-11 relative).
    The output is written to DRAM as float16 (the harness tolerance is
    2e-2; f16 output rounding adds ~3e-4) — this HALVES the output HBM
    traffic, which is the binding roofline of the whole kernel.
    Per-core inputs: ast = stacked A^T slice [32, ILOC*128] f16,
                     bst = stacked G^T [32, W] f16.
    """
    f16 = mybir.dt.float16
    nc = bacc.Bacc()
    ast = nc.declare_dram_parameter("ast", [KST, ILOC * BC], f16, isOutput=False)
    bst = nc.declare_dram_parameter("bst", [KST, W], f16, isOutput=False)
    if _alt:  # test-harness only: alternate rhs across repeats so no repeat
        bst2 = nc.declare_dram_parameter("bst2", [KST, W], f16, isOutput=False)
    out = nc.declare_dram_parameter("out", [BC, SHARD], f16, isOutput=True)

    with TileContext(nc) as tc:
        with (
            tc.tile_pool(name="cpool", bufs=1) as cpool,
            tc.tile_pool(name="opool", bufs=o_bufs) as opool,
            tc.tile_pool(name="ppool", bufs=p_bufs // PG, space="PSUM") as ppool,
        ):
            bsttile = cpool.tile([KST, W], f16)
            nc.sync.dma_start(out=bsttile[:, :], in_=bst[:, :])
            if _alt:
                bsttile2 = cpool.tile([KST, W], f16)
                nc.sync.dma_start(out=bsttile2[:, :], in_=bst2[:, :])
            asttile = cpool.tile([KST, ILOC * BC], f16)
            # progressive chunks: a tiny first chunk ungates the first
            # matmuls almost immediately; a few big tails amortize the
            # ~0.6 us per-DMA-instruction overhead that otherwise
            # dominates the kernel head (16 uniform chunks cost ~11 us).
            # Chunks go out on different engines' DGE queues so their
            # descriptor-gen + transfers overlap instead of serializing
            # on the SP queue.
            bounds = [0, 4, 16, 40, 64]
            engines = [nc.sync, nc.scalar, nc.sync, nc.scalar]
            for (c0, c1), eng in zip(zip(bounds[:-1], bounds[1:]), engines):
                eng.dma_start(
                    out=asttile[:, c0 * BC : c1 * BC],
                    in_=ast[:, c0 * BC : c1 * BC],
                )
            # smaller first and last groups: the first output DMA fires
            # after 4 i-rows instead of 8 (earlier pipeline start), and the
            # final drain carries half the tail
            groups = [OG // 2] + [OG] * (ILOC // OG - 1) + [OG // 2]
            starts = [sum(groups[:i]) for i in range(len(groups))]
            for _rep in range(repeats):
                bt_use = bsttile2 if (_alt and _rep % 2) else bsttile
                for g0, gw in zip(starts, groups):
                    otile = opool.tile([128, gw * W], f16)
                    for s2 in range(gw // PG):
                        ptile = ppool.tile([128, PG * W], mybir.dt.float32)
                        for u in range(PG):
                            il = g0 + s2 * PG + u
                            nc.tensor.matmul(
                                ptile[:, u * W : (u + 1) * W],
                                lhsT=asttile[:, il * BC : (il + 1) * BC],
                                rhs=bt_use[:, :],
                                start=True,
                                stop=True,
                            )
                        # split the PSUM->SBUF copy (with f32->f16 cast)
                        # across VectorE and ScalarE in proportion to their
                        # fp32-input rates (123 vs 154 G elem/s — DVE's 2x
                        # 16-bit mode does not apply to PSUM/f32 reads)
                        vshare = (PG * W * 9) // 20
                        nc.vector.tensor_copy(
                            otile[:, s2 * PG * W : s2 * PG * W + vshare],
                            ptile[:, :vshare],
                        )
                        nc.scalar.copy(
                            otile[:, s2 * PG * W + vshare : (s2 + 1) * PG * W],
                            ptile[:, vshare:],
                        )
                    # two half-tile stores: spreads work over more DMA
                    # engine slots and lets the first half fly while the
                    # second half's copies finish
                    base = g0 * W
                    half = gw * W // 2
                    nc.sync.dma_start(
                        out=out[:, base : base + half],
                        in_=otile[:, :half],
                    )
                    nc.sync.dma_start(
                        out=out[:, base + half : base + gw * W],
                        in_=otile[:, half:],
                    )
    nc.finalize()
    return nc


def _try_separate(basis4):
    """If basis[(i,j),p,q] == F[i,p] * G[j,q] (to fp32 accuracy), return
    (F, G) as float64 arrays; else None.  Exact-by-construction check: the
    factorization is verified elementwise against the provided data."""
    S = basis4.sum(axis=(1, 3), dtype=np.float64)  # [H, M1] = F * sum(G)
    T = basis4.sum(axis=(0, 2), dtype=np.float64)  # [W, N1] = G * sum(F)
    tot = float(S.sum())
    if not np.isfinite(tot) or abs(tot) < 1e-30:
        return None
    F = S
    G = T / tot
    scale = float(np.max(np.abs(basis4)))
    if scale == 0.0 or not np.isfinite(scale):
        return None
    # chunked elementwise verification of the reconstruction.  A truly
    # separable f32 tensor reconstructs to ~3e-8 * scale (f32 rounding);
    # 1e-6 leaves margin while rejecting anything meaningfully non-rank-1.
    for i0 in range(0, H, 64):
        rec = np.einsum(
            "ip,jq->ijpq", F[i0 : i0 + 64], G, optimize=True
        ).astype(np.float32)
        err = np.max(np.abs(rec - basis4[i0 : i0 + 64]))
        if not (err <= 1e-6 * scale):
            return None
    return F, G


def kernel(K: np.ndarray, basis: np.ndarray) -> np.ndarray:
    global LAST_RESULT
    K = np.ascontiguousarray(np.asarray(K, dtype=np.float32))
    basis = np.asarray(basis, dtype=np.float32)

    force = os.environ.get("BASS_KERNEL_FORCE", "")  # "", "fast", "general"
    fact = None
    if force != "general":
        fact = _try_separate(basis.reshape(H, W, M1, N1))

    trace = os.environ.get("BASS_KERNEL_TRACE", "0") == "1"
    core_ids = list(range(N_CORES))

    if fact is not None:
        try:
            return _run_fast(K, fact, core_ids, trace)
        except Exception:
            pass  # graceful degradation: fall through to the general path
    return _run_general(K, basis, core_ids, trace)


def _build_nc_noop():
    """Timing-harness helper: a NEFF with the fast path's exact I/O
    signature but ~zero device work (one tiny load + one tiny store).
    Launching it interleaved with the real reps=1 kernel and taking the
    paired wall-clock difference measures the real kernel's full
    single-exec device time directly on hardware — head, steady state,
    and drain — with the axon dispatch floor cancelled."""
    f16 = mybir.dt.float16
    nc = bacc.Bacc()
    ast = nc.declare_dram_parameter("ast", [KST, ILOC * BC], f16, isOutput=False)
    bst = nc.declare_dram_parameter("bst", [KST, W], f16, isOutput=False)
    out = nc.declare_dram_parameter("out", [BC, SHARD], f16, isOutput=True)
    with TileContext(nc) as tc:
        with tc.tile_pool(name="cpool", bufs=1) as cpool:
            t = cpool.tile([KST, W], f16)
            nc.sync.dma_start(out=t[:, :], in_=bst[:, :])
            t2 = cpool.tile([KST, W], f16)
            nc.sync.dma_start(out=t2[:, :], in_=ast[:, :W])
            nc.sync.dma_start(out=out[:KST, :W], in_=t[:, :])
            nc.sync.dma_start(out=out[KST : 2 * KST, :W], in_=t2[:, :])
    nc.finalize()
    return nc


def _pack_fast_inputs(K, fact):
    """Host-side packing for the fast path: per-core input maps."""
    F, G = fact
    # rebalance so both factors are O(1): the f16 hi/lo split loses
    # precision badly when one factor carries a ~512x scale
    c = float(np.max(np.abs(F)))
    F = F / c
    G = G * c
    # A[bc, i, q] = sum_p F[i,p] * K[bc,p,q]
    A = np.einsum(
        "ip,bpq->biq", F, K.reshape(BC, M1, N1).astype(np.float64), optimize=True
    ).astype(np.float32)
    G32 = G.astype(np.float32)
    bh = G32.astype(np.float16)
    bst = np.concatenate([bh.T, bh.T], axis=0)  # [32, W]
    bst = np.ascontiguousarray(bst)
    A_hi = A.astype(np.float16)
    A_lo = (A - A_hi.astype(np.float32)).astype(np.float16)
    in_maps = []
    for i in range(N_CORES):
        sl = slice(i * ILOC, (i + 1) * ILOC)
        aht = A_hi[:, sl, :].transpose(2, 1, 0).reshape(M1, ILOC * BC)
        alt = A_lo[:, sl, :].transpose(2, 1, 0).reshape(M1, ILOC * BC)
        ast = np.ascontiguousarray(
            np.concatenate([aht, alt], axis=0)
        )  # [32, ILOC*BC]
        in_maps.append({"ast": ast, "bst": bst})
    return in_maps


def _run_fast(K, fact, core_ids, trace):
    global LAST_RESULT
    in_maps = _pack_fast_inputs(K, fact)
    nc = _build_nc_fast()
    LAST_RESULT = run_bass_kernel_spmd(nc, in_maps, core_ids=core_ids, trace=trace)
    res = LAST_RESULT.results
    out = np.concatenate(
        [res[i]["out"] for i in range(N_CORES)], axis=1
    )  # [128, HW] f16
    return out.astype(np.float32).reshape(1, B, C, H, W)


def _run_general(K, basis, core_ids, trace):
    global LAST_RESULT
    kt_full = np.ascontiguousarray(K.reshape(BC, KDIM).T)  # [256, 128]
    bflat = basis.reshape(HW, KDIM)
    in_maps = []
    for i in range(N_CORES):
        bt_i = np.ascontiguousarray(
            bflat[i * SHARD : (i + 1) * SHARD].T
        )  # [256, SHARD]
        in_maps.append({"kt": kt_full, "bt": bt_i})
    nc = _build_nc(nt=1024, b_bufs=4, o_bufs=4, p_bufs=2)
    LAST_RESULT = run_bass_kernel_spmd(nc, in_maps, core_ids=core_ids, trace=trace)
    res = LAST_RESULT.results
    out = np.concatenate([res[i]["out"] for i in range(N_CORES)], axis=1)  # [128, HW]
    return out.reshape(1, B, C, H, W)

